# revision 11
# baseline (speedup 1.0000x reference)
"""Two-layer GCN (PyG GCNConv x2 + ReLU) on 8 Trainium2 NeuronCores.

Strategy (dst-sharded SPMD, aggregation-first):
  - GCN layer = relu((A_hat @ x) @ W + b): the dense matmul commutes with the
    aggregation, so each layer gathers rows of the (bf16) node table, scatter-
    adds them via on-device-built selection-matrix matmuls into per-block
    PSUM, then applies the 128x128 dense + ReLU per 128-node block.
  - Layer 1's table is x itself (replicated input) -> no dense pre-pass and no
    collective before layer 1. Self-loop contributions bypass the gather
    entirely: they are a host-prescaled table xts = a^2 * x applied as a
    second accumulating matmul into the dense PSUM.
  - Layer 2's table is out1, exchanged via 4 chunked AllGathers (25 blocks
    each) pipelined against edge-phase compute: chunk c is gathered by layer 2
    as soon as AG_c lands while later chunks are still being produced.
  - Layer-1 tiles run (chunk, window)-major with a bf16 SBUF accumulator so
    each gather window is one long dma_gather run (fewer SWDGE calls); per-
    window PSUM partials are ACT-evicted to bf16 and combined on the DVE in
    its all-16-bit fast mode.
  - Selection matrices S[e, d] = norm_e * (d == dst_e) are built on the DVE
    per tile from compact per-edge metadata (one fused is_equal*mult
    tensor_scalar against a constant bf16 iota tile, f32 scalars to keep the
    DVE 2x 16-bit mode) -- no S-matrix DMA from HBM.
  - Everything bf16 except PSUM accumulation (f32), the layer-2 inter-chunk
    accumulator (f32), and the epilogue math; output written bf16 and
    upcast to f32 on host.
  - Nodes are padded to 102400 and dealt round-robin over the 800
    (core, block) pairs by descending in-degree so per-block edge counts are
    balanced; all device-side structures live in position space.
"""

import numpy as np

import concourse.bass as bass
import concourse.bacc as bacc
import concourse.mybir as mybir
import concourse.tile as tile
from concourse.bass_utils import run_bass_kernel_spmd

N = 100000
E = 640000
D = 128
NCORES = 8
NPAD = 102400
SHARD = NPAD // NCORES        # 12800
NBLK = SHARD // 128           # 100 dst blocks per core
WIN = 25600                   # gather window rows (int16-safe); NPAD/WIN = 4
NWIN = NPAD // WIN            # 4
NCHUNK = 4                    # AllGather chunks for the layer-2 table
CBLK = NBLK // NCHUNK         # 25 blocks per chunk
SUB = 5                       # blocks per layer-1 PSUM sub-chunk (banks)
NSUB = CBLK // SUB            # 5 sub-chunks per chunk
CHUNK_T = 8                   # tiles (128 edges) per dma_gather call

BF = mybir.dt.bfloat16
NPBF = mybir.dt.np(BF)

_CACHE = {}
PADNEG = False   # pad gather slots with -1 (DMA skips them) instead of 0 (crashes HW; keep False)
QUEUES = 4       # SWDGE queues for gather round-robin
SCRATCH = None   # dynamic_dma_scratch_size override


def _group_schedule(gid, rel, nrm, dloc, ngroups, counts_max):
    """Common-tile-schedule packing for one layer on one core.

    Returns flat (per 128-slot tile) gidx/norm/dst arrays in tile order.
    """
    T = (counts_max + 127) // 128
    T = np.maximum(T, 1)
    tile_base = np.zeros(ngroups + 1, np.int64)
    tile_base[1:] = np.cumsum(T)
    t_total = int(tile_base[-1])

    order = np.lexsort((rel, gid))
    gid_s = gid[order]
    counts = np.bincount(gid_s, minlength=ngroups)
    grp_off = np.zeros(ngroups + 1, np.int64)
    grp_off[1:] = np.cumsum(counts)
    rank = np.arange(gid_s.shape[0], dtype=np.int64) - grp_off[gid_s]
    pos = tile_base[gid_s] * 128 + rank

    gidx = np.full(t_total * 128, -1 if PADNEG else 0, np.int16)
    norm = np.zeros(t_total * 128, np.float32)
    dst = np.zeros(t_total * 128, np.int64)
    gidx[pos] = rel[order].astype(np.int16)
    norm[pos] = nrm[order]
    dst[pos] = dloc[order]
    return T, tile_base, t_total, gidx, norm, dst


def _wrap_idx(gidx, calls, t_total):
    idxw = np.zeros((128, t_total * 8), np.int16)
    for (w, t0, nt) in calls:
        blk = gidx[t0 * 128:(t0 + nt) * 128].reshape(nt * 8, 16).T
        idxw[:, t0 * 8:(t0 + nt) * 8] = np.tile(blk, (8, 1))
    return idxw


def _calls_from_runs(tile_base, runs):
    """runs: list of (window, gid_start, gid_end). Chop each run's tile range
    into dma_gather calls of <= CHUNK_T tiles."""
    calls = []
    for (w, g0, g1) in runs:
        t = int(tile_base[g0])
        t_end = int(tile_base[g1])
        while t < t_end:
            nt = min(CHUNK_T, t_end - t)
            calls.append((w, t, nt))
            t += nt
    return calls


def _host_prep(x, edge_index, W1, b1, W2, b2):
    x = np.asarray(x, dtype=np.float32)
    ei = np.asarray(edge_index)
    W1 = np.asarray(W1, dtype=np.float32)
    W2 = np.asarray(W2, dtype=np.float32)
    b1 = np.asarray(b1, dtype=np.float32)
    b2 = np.asarray(b2, dtype=np.float32)
    n = x.shape[0]

    src = np.concatenate([ei[0].astype(np.int64), np.arange(n, dtype=np.int64)])
    dst = np.concatenate([ei[1].astype(np.int64), np.arange(n, dtype=np.int64)])
    deg = np.bincount(dst, minlength=NPAD).astype(np.float32)
    a = np.zeros(NPAD, np.float32)
    nz = deg > 0
    a[nz] = 1.0 / np.sqrt(deg[nz])

    # degree-balanced node->position permutation (nodes dealt round-robin over
    # the 800 (core, block) pairs by descending degree)
    order_by_deg = np.argsort(-deg, kind="stable")
    i = np.arange(NPAD, dtype=np.int64)
    cb = i % (NCORES * NBLK)
    position_of_rank = (cb % NCORES) * SHARD + (cb // NCORES) * 128 + i // (NCORES * NBLK)
    pos_of_node = np.empty(NPAD, np.int64)
    pos_of_node[order_by_deg] = position_of_rank
    node_at_pos = np.empty(NPAD, np.int64)
    node_at_pos[pos_of_node] = i

    ps = pos_of_node[src]
    pd = pos_of_node[dst]
    core = pd // SHARD
    norm_all = a[src] * a[dst]
    is_self = np.zeros(src.shape[0], bool)
    is_self[E:] = True          # the appended self-loops

    x_pad = np.zeros((NPAD, D), np.float32)
    x_pad[:n] = x
    x_perm = x_pad[node_at_pos]
    x_perm_bf = np.ascontiguousarray(x_perm.astype(NPBF))
    a_pos = a[node_at_pos]
    # per-position self-loop contribution table: x[d] * a_d^2, feature-major
    xts_full = (x_perm * (a_pos ** 2)[:, None]).astype(NPBF)

    NG1 = NCHUNK * NSUB * NWIN * SUB     # 400
    NG2 = NCHUNK * NBLK                  # 400

    per_core = []
    cmax1 = np.zeros(NG1, np.int64)
    cmax2 = np.zeros(NG2, np.int64)
    for k in range(NCORES):
        m = core == k
        m1 = m & ~is_self               # layer 1 skips self-loops (handled densely)
        s1k = ps[m1]
        d1k = pd[m1]
        nrm1 = norm_all[m1]
        blk1 = (d1k % SHARD) // 128
        dloc1 = d1k % 128
        c1 = blk1 // CBLK
        ss1 = (blk1 % CBLK) // SUB
        b5 = blk1 % SUB
        w1 = s1k // WIN
        gid1 = ((c1 * NWIN + w1) * NSUB + ss1) * SUB + b5
        rel1 = s1k - w1 * WIN

        s_k = ps[m]
        d_k = pd[m]
        nrm = norm_all[m]
        blk = (d_k % SHARD) // 128
        dloc = d_k % 128
        sb = (s_k % SHARD) // 128        # source block 0..99
        c2 = sb // CBLK
        rel2 = (s_k // SHARD) * (CBLK * 128) + (sb % CBLK) * 128 + (s_k % 128)
        gid2 = c2 * NBLK + blk

        cmax1 = np.maximum(cmax1, np.bincount(gid1, minlength=NG1))
        cmax2 = np.maximum(cmax2, np.bincount(gid2, minlength=NG2))
        per_core.append((gid1, rel1, nrm1, dloc1, gid2, rel2, nrm, dloc))

    T1 = (cmax1 + 127) // 128
    T1 = np.maximum(T1, 1)
    tb1 = np.zeros(NG1 + 1, np.int64)
    tb1[1:] = np.cumsum(T1)
    t1 = int(tb1[-1])
    T2 = (cmax2 + 127) // 128
    T2 = np.maximum(T2, 1)
    tb2 = np.zeros(NG2 + 1, np.int64)
    tb2[1:] = np.cumsum(T2)
    t2 = int(tb2[-1])

    # gather-call schedule (identical across cores)
    runs1 = []
    for c in range(NCHUNK):
        for w in range(NWIN):
            g0 = (c * NWIN + w) * NSUB * SUB
            runs1.append((w, g0, g0 + NSUB * SUB))
    calls1 = _calls_from_runs(tb1, runs1)
    runs2 = [(c, c * NBLK, (c + 1) * NBLK) for c in range(NCHUNK)]
    calls2 = _calls_from_runs(tb2, runs2)

    in_maps = []
    b1nz = bool(np.any(b1 != 0.0))
    b2nz = bool(np.any(b2 != 0.0))
    for k in range(NCORES):
        gid1, rel1, nrm1, dloc1, gid2, rel2, nrm, dloc = per_core[k]
        _, _, _, gidx1, norm1, dst1 = _group_schedule(
            gid1, rel1, nrm1, dloc1, NG1, cmax1)
        _, _, _, gidx2, norm2, dst2 = _group_schedule(
            gid2, rel2, nrm, dloc, NG2, cmax2)
        im = {
            "x": x_perm_bf,
            "xts": np.ascontiguousarray(
                xts_full[k * SHARD:(k + 1) * SHARD].T),
            "idx1": _wrap_idx(gidx1, calls1, t1),
            "md1": np.ascontiguousarray(dst1.reshape(t1, 128).T.astype(np.float32)),
            "mn1": np.ascontiguousarray(norm1.reshape(t1, 128).T.astype(np.float32)),
            "idx2": _wrap_idx(gidx2, calls2, t2),
            "md2": np.ascontiguousarray(dst2.reshape(t2, 128).T.astype(np.float32)),
            "mn2": np.ascontiguousarray(norm2.reshape(t2, 128).T.astype(np.float32)),
            "W1": W1.astype(NPBF),
            "W2": W2.astype(NPBF),
            "b1bc": np.broadcast_to(b1, (128, 128)).astype(np.float32).copy(),
            "b2bc": np.broadcast_to(b2, (128, 128)).astype(np.float32).copy(),
        }
        in_maps.append(im)

    sched = {
        "T1": tuple(int(v) for v in T1),
        "T2": tuple(int(v) for v in T2),
        "calls1": tuple(calls1),
        "calls2": tuple(calls2),
        "t1": t1,
        "t2": t2,
        "b1nz": b1nz,
        "b2nz": b2nz,
    }
    return in_maps, sched, pos_of_node


def _build_program(sched, variant="full"):
    T1 = np.array(sched["T1"], np.int64)
    T2 = np.array(sched["T2"], np.int64)
    tb1 = np.zeros(T1.shape[0] + 1, np.int64)
    tb1[1:] = np.cumsum(T1)
    tb2 = np.zeros(T2.shape[0] + 1, np.int64)
    tb2[1:] = np.cumsum(T2)
    calls1 = list(sched["calls1"])
    calls2 = list(sched["calls2"])
    t1, t2 = sched["t1"], sched["t2"]
    b1nz, b2nz = sched["b1nz"], sched["b2nz"]

    # per-tile annotations, layer 1: block id + first/last flags
    # gid1 -> (c, s, w, b5); block = (c*NSUB + s)*SUB + b5
    blk_of_t1 = np.zeros(t1, np.int64)
    w_of_t1 = np.zeros(t1, np.int64)
    first_t1 = np.zeros(t1, bool)
    last_t1 = np.zeros(t1, bool)
    for g in range(T1.shape[0]):
        b5 = g % SUB
        s = (g // SUB) % NSUB
        w = (g // (SUB * NSUB)) % NWIN
        c = g // (SUB * NSUB * NWIN)
        b = (c * NSUB + s) * SUB + b5
        blk_of_t1[tb1[g]:tb1[g + 1]] = b
        w_of_t1[tb1[g]:tb1[g + 1]] = w
        first_t1[tb1[g]] = True
        last_t1[tb1[g + 1] - 1] = True
    # layer 2: gid2 = c*NBLK + b
    blk_of_t2 = np.zeros(t2, np.int64)
    c_of_t2 = np.zeros(t2, np.int64)
    first_t2 = np.zeros(t2, bool)
    last_t2 = np.zeros(t2, bool)
    for g in range(T2.shape[0]):
        c = g // NBLK
        b = g % NBLK
        blk_of_t2[tb2[g]:tb2[g + 1]] = b
        c_of_t2[tb2[g]:tb2[g + 1]] = c
        first_t2[tb2[g]] = True
        last_t2[tb2[g + 1] - 1] = True

    kw = {}
    if SCRATCH is not None:
        kw["dynamic_dma_scratch_size"] = SCRATCH
    nc = bacc.Bacc("TRN2", target_bir_lowering=False, debug=False,
                   num_devices=NCORES, num_swdge_queues=QUEUES, **kw)
    f32 = mybir.dt.float32
    i16 = mybir.dt.int16

    x_d = nc.dram_tensor("x", [NPAD, D], BF, kind="ExternalInput")
    xts_d = nc.dram_tensor("xts", [D, SHARD], BF, kind="ExternalInput")
    idx1_d = nc.dram_tensor("idx1", [128, t1 * 8], i16, kind="ExternalInput")
    md1_d = nc.dram_tensor("md1", [128, t1], f32, kind="ExternalInput")
    mn1_d = nc.dram_tensor("mn1", [128, t1], f32, kind="ExternalInput")
    idx2_d = nc.dram_tensor("idx2", [128, t2 * 8], i16, kind="ExternalInput")
    md2_d = nc.dram_tensor("md2", [128, t2], f32, kind="ExternalInput")
    mn2_d = nc.dram_tensor("mn2", [128, t2], f32, kind="ExternalInput")
    W1_d = nc.dram_tensor("W1", [D, D], BF, kind="ExternalInput")
    W2_d = nc.dram_tensor("W2", [D, D], BF, kind="ExternalInput")
    b1_d = nc.dram_tensor("b1bc", [128, 128], f32, kind="ExternalInput")
    b2_d = nc.dram_tensor("b2bc", [128, 128], f32, kind="ExternalInput")
    out_d = nc.dram_tensor("out", [SHARD, D], BF, kind="ExternalOutput")

    ob = [nc.dram_tensor(f"ob{c}", [CBLK * 128, D], BF) for c in range(NCHUNK)]
    hf = [nc.dram_tensor(f"hf{c}", [CBLK * 128 * NCORES, D], BF,
                         addr_space="Shared") for c in range(NCHUNK)]

    with tile.TileContext(nc) as tc:
        with (
            tc.tile_pool(name="const", bufs=1) as p_const,
            tc.tile_pool(name="acc", bufs=1) as p_acc,
            tc.tile_pool(name="msg", bufs=8) as p_msg,
            tc.tile_pool(name="sel", bufs=8) as p_sel,
            tc.tile_pool(name="agg", bufs=4) as p_agg,
            tc.tile_pool(name="o1", bufs=4) as p_o1,
            tc.tile_pool(name="tmp", bufs=2) as p_tmp,
            tc.tile_pool(name="eps", bufs=SUB, space="PSUM") as p_eps,
            tc.tile_pool(name="dps", bufs=2, space="PSUM") as p_dps,
        ):
            W1_t = p_const.tile([D, D], BF)
            W2_t = p_const.tile([D, D], BF)
            idx1_t = p_const.tile([128, t1 * 8], i16)
            md1_t = p_const.tile([128, t1], f32)
            mn1_t = p_const.tile([128, t1], f32)
            idx2_t = p_const.tile([128, t2 * 8], i16)
            md2_t = p_const.tile([128, t2], f32)
            mn2_t = p_const.tile([128, t2], f32)
            nc.sync.dma_start(out=idx1_t[:], in_=idx1_d[:])
            nc.sync.dma_start(out=md1_t[:], in_=md1_d[:])
            nc.sync.dma_start(out=mn1_t[:], in_=mn1_d[:])
            nc.sync.dma_start(out=W1_t[:], in_=W1_d[:])
            nc.sync.dma_start(out=W2_t[:], in_=W2_d[:])
            if b1nz or b2nz:
                b1_t = p_const.tile([128, 128], f32)
                b2_t = p_const.tile([128, 128], f32)
                nc.sync.dma_start(out=b1_t[:], in_=b1_d[:])
                nc.sync.dma_start(out=b2_t[:], in_=b2_d[:])

            xts_t = p_const.tile([D, SHARD], BF)
            iota_i = p_const.tile([128, 128], mybir.dt.int32)
            iota_bf = p_const.tile([128, 128], BF)
            nc.gpsimd.iota(out=iota_i[:], pattern=[[1, 128]], base=0,
                           channel_multiplier=0)
            nc.vector.tensor_copy(out=iota_bf[:], in_=iota_i[:])

            acc1 = p_acc.tile([128, SHARD], BF, name="acc1")
            acc2 = p_acc.tile([128, SHARD], BF)

            if PADNEG:
                for _z in range(8):
                    zt = p_msg.tile([128, CHUNK_T, D], BF, tag="msg", name="zmsg")
                    nc.vector.memset(zt[:], 0.0)

            def build_S(md_t, mn_t, t):
                S_t = p_sel.tile([128, 128], BF, tag="sel")
                nc.vector.tensor_scalar(
                    out=S_t[:], in0=iota_bf[:],
                    scalar1=md_t[:, t:t + 1], scalar2=mn_t[:, t:t + 1],
                    op0=mybir.AluOpType.is_equal, op1=mybir.AluOpType.mult)
                return S_t

            # ---------------- layer 1 ----------------
            pending = {}
            stage1 = [None]
            stage2 = [None]
            ag_ready = []
            chunk_done_emitted = [False] * NCHUNK
            def emit_ags():
                while ag_ready:
                    cr = ag_ready.pop(0)
                    nc.gpsimd.collective_compute(
                        "AllGather", mybir.AluOpType.bypass,
                        replica_groups=[list(range(NCORES))],
                        ins=[ob[cr][:]], outs=[hf[cr][:]],
                    )

            last_call_chunk = [-1, 0]
            for ci, (w, t0, nt) in enumerate(calls1):
                call_chunk = int(blk_of_t1[t0]) // CBLK
                if call_chunk != last_call_chunk[0]:
                    last_call_chunk[0] = call_chunk
                    last_call_chunk[1] = 0
                else:
                    last_call_chunk[1] += 1
                    if last_call_chunk[1] == 4:
                        emit_ags()
                if ci == 2:
                    nc.sync.dma_start(out=xts_t[:], in_=xts_d[:])
                if call_chunk == 2 and last_call_chunk[1] == 1:
                    nc.sync.dma_start(out=idx2_t[:], in_=idx2_d[:])
                    nc.sync.dma_start(out=md2_t[:], in_=md2_d[:])
                    nc.sync.dma_start(out=mn2_t[:], in_=mn2_d[:])
                msg_t = p_msg.tile([128, CHUNK_T, D], BF, tag="msg")
                nc.gpsimd.dma_gather(
                    out_ap=msg_t[:, :nt, :],
                    in_ap=x_d[w * WIN:(w + 1) * WIN, :],
                    idxs_ap=idx1_t[:, t0 * 8:(t0 + nt) * 8],
                    num_idxs=nt * 128, num_idxs_reg=nt * 128,
                    elem_size=D, queue_num=ci % QUEUES)
                for t in range(t0, t0 + nt):
                    b = int(blk_of_t1[t])
                    w_t = int(w_of_t1[t])
                    S_t = build_S(md1_t, mn1_t, t)
                    if first_t1[t]:
                        pending[b] = p_eps.tile([128, D], f32, space="PSUM",
                                                tag="eps", name="eps")
                    ps = pending[b]
                    nc.tensor.matmul(out=ps[:], lhsT=msg_t[:, t - t0, :],
                                     rhs=S_t[:], start=bool(first_t1[t]),
                                     stop=bool(last_t1[t]))
                    if last_t1[t] and w_t < NWIN - 1:
                        # evict this window's partial into the bf16 accumulator
                        del pending[b]
                        a1blk = acc1[:, b * 128:(b + 1) * 128]
                        if w_t == 0:
                            nc.scalar.activation(
                                out=a1blk, in_=ps[:],
                                func=mybir.ActivationFunctionType.Copy)
                        else:
                            tb_ = p_agg.tile([128, D], BF, tag="agg",
                                             name="tb1")
                            nc.scalar.activation(
                                out=tb_[:], in_=ps[:],
                                func=mybir.ActivationFunctionType.Copy)
                            nc.vector.tensor_add(out=a1blk, in0=a1blk,
                                                 in1=tb_[:])
                    elif last_t1[t]:
                        del pending[b]
                        tb_ = p_agg.tile([128, D], BF, tag="agg", name="tb1")
                        nc.scalar.activation(
                            out=tb_[:], in_=ps[:],
                            func=mybir.ActivationFunctionType.Copy)
                        aggbf = p_agg.tile([128, D], BF, tag="agg")
                        nc.vector.tensor_add(out=aggbf[:],
                                             in0=acc1[:, b * 128:(b + 1) * 128],
                                             in1=tb_[:])
                        ps2 = p_dps.tile([128, D], f32, space="PSUM", tag="dps")
                        nc.tensor.matmul(out=ps2[:], lhsT=aggbf[:], rhs=W1_t[:],
                                         start=True, stop=False)
                        nc.tensor.matmul(
                            out=ps2[:], lhsT=xts_t[:, b * 128:(b + 1) * 128],
                            rhs=W1_t[:], start=False, stop=True)
                        b5 = b % SUB
                        if b5 == 0:
                            stage1[0] = p_o1.tile([128, SUB * D], BF,
                                                  tag="o1", name="o1w")
                        o1 = stage1[0][:, b5 * D:(b5 + 1) * D]
                        if b1nz:
                            tmp = p_tmp.tile([128, D], f32, tag="tmp")
                            nc.vector.tensor_add(out=tmp[:], in0=ps2[:],
                                                 in1=b1_t[:])
                            nc.scalar.activation(
                                out=o1, in_=tmp[:],
                                func=mybir.ActivationFunctionType.Relu)
                        else:
                            nc.scalar.activation(
                                out=o1, in_=ps2[:],
                                func=mybir.ActivationFunctionType.Relu)
                        c = b // CBLK
                        if b5 == SUB - 1:
                            sl = (b % CBLK) // SUB
                            nc.sync.dma_start(
                                out=ob[c][sl * SUB * 128:(sl + 1) * SUB * 128,
                                          :].rearrange("(j r) f -> r j f",
                                                       j=SUB),
                                in_=stage1[0][:])
                        # mark chunk c ready; the AllGather is emitted after
                        # the NEXT chunk's gather calls so the Pool queue
                        # keeps generating descriptors while AG_c waits on
                        # the bounce writes
                        if (b % CBLK) == CBLK - 1 and variant == "full":
                            assert not chunk_done_emitted[c]
                            chunk_done_emitted[c] = True
                            ag_ready.append(c)

            if variant == "full":
                emit_ags()

            # ---------------- layer 2 ----------------
            pending2 = {}
            for ci, (c, t0, nt) in enumerate(calls2):
                msg_t = p_msg.tile([128, CHUNK_T, D], BF, tag="msg")
                src_ap = hf[c][:] if variant == "full" else x_d[0:CBLK * 128 * NCORES, :]
                nc.gpsimd.dma_gather(
                    out_ap=msg_t[:, :nt, :],
                    in_ap=src_ap,
                    idxs_ap=idx2_t[:, t0 * 8:(t0 + nt) * 8],
                    num_idxs=nt * 128, num_idxs_reg=nt * 128,
                    elem_size=D, queue_num=ci % QUEUES)
                for t in range(t0, t0 + nt):
                    b = int(blk_of_t2[t])
                    S_t = build_S(md2_t, mn2_t, t)
                    if first_t2[t]:
                        pending2[b] = p_eps.tile([128, D], f32, space="PSUM",
                                                 tag="eps", name="eps")
                    ps = pending2[b]
                    nc.tensor.matmul(out=ps[:], lhsT=msg_t[:, t - t0, :],
                                     rhs=S_t[:], start=bool(first_t2[t]),
                                     stop=bool(last_t2[t]))
                    if last_t2[t]:
                        del pending2[b]
                        accblk = acc2[:, b * 128:(b + 1) * 128]
                        if c == 0:
                            nc.scalar.activation(
                                out=accblk, in_=ps[:],
                                func=mybir.ActivationFunctionType.Copy)
                        elif c < NCHUNK - 1:
                            # ACT evicts PSUM to bf16 so the DVE add runs in
                            # the all-16-bit fast mode
                            tbf = p_agg.tile([128, D], BF, tag="agg",
                                             name="tbf")
                            nc.scalar.activation(
                                out=tbf[:], in_=ps[:],
                                func=mybir.ActivationFunctionType.Copy)
                            nc.vector.tensor_add(out=accblk, in0=accblk,
                                                 in1=tbf[:])
                        else:
                            tbf = p_agg.tile([128, D], BF, tag="agg",
                                             name="tbf")
                            nc.scalar.activation(
                                out=tbf[:], in_=ps[:],
                                func=mybir.ActivationFunctionType.Copy)
                            agg2 = p_agg.tile([128, D], BF, tag="agg")
                            nc.vector.tensor_add(out=agg2[:], in0=accblk,
                                                 in1=tbf[:])
                            ps2 = p_dps.tile([128, D], f32, space="PSUM",
                                             tag="dps")
                            nc.tensor.matmul(out=ps2[:], lhsT=agg2[:],
                                             rhs=W2_t[:], start=True, stop=True)
                            b5 = b % SUB
                            if b5 == 0:
                                stage2[0] = p_o1.tile([128, SUB * D], BF,
                                                      tag="o1", name="otw")
                            ot = stage2[0][:, b5 * D:(b5 + 1) * D]
                            if b2nz:
                                tmp = p_tmp.tile([128, D], f32, tag="tmp")
                                nc.vector.tensor_add(out=tmp[:], in0=ps2[:],
                                                     in1=b2_t[:])
                                nc.scalar.activation(
                                    out=ot, in_=tmp[:],
                                    func=mybir.ActivationFunctionType.Relu)
                            else:
                                nc.scalar.activation(
                                    out=ot, in_=ps2[:],
                                    func=mybir.ActivationFunctionType.Relu)
                            if b5 == SUB - 1:
                                nc.sync.dma_start(
                                    out=out_d[(b - SUB + 1) * 128:
                                              (b + 1) * 128, :].rearrange(
                                        "(j r) f -> r j f", j=SUB),
                                    in_=stage2[0][:])

    nc.compile()
    return nc


def prepare(x, edge_index, W1, b1, W2, b2, variant="full"):
    in_maps, sched, pos_of_node = _host_prep(x, edge_index, W1, b1, W2, b2)
    key = (sched["T1"], sched["T2"], sched["calls1"], sched["calls2"],
           sched["b1nz"], sched["b2nz"], variant, PADNEG, QUEUES, SCRATCH)
    if key not in _CACHE:
        _CACHE[key] = _build_program(sched, variant)
    return _CACHE[key], in_maps, pos_of_node


def kernel(x, edge_index, W1, b1, W2, b2):
    nc, in_maps, pos_of_node = prepare(x, edge_index, W1, b1, W2, b2)
    res = run_bass_kernel_spmd(nc, in_maps, list(range(NCORES)))
    full = np.concatenate([res.results[k]["out"] for k in range(NCORES)], axis=0)
    n = np.asarray(x).shape[0]
    return np.ascontiguousarray(full[pos_of_node[:n]].astype(np.float32))


# revision 12
# speedup vs baseline: 1.9852x; 1.9852x over previous
"""Two-layer GCN (PyG GCNConv x2 + ReLU) on 8 Trainium2 NeuronCores.

Strategy (dst-sharded SPMD, aggregation-first):
  - GCN layer = relu((A_hat @ x) @ W + b): the dense matmul commutes with the
    aggregation, so each layer gathers rows of the (bf16) node table, scatter-
    adds them via on-device-built selection-matrix matmuls into per-block
    PSUM, then applies the 128x128 dense + ReLU per 128-node block.
  - Layer 1's table is x itself (replicated input) -> no dense pre-pass and no
    collective before layer 1. Self-loop contributions bypass the gather
    entirely: they are a host-prescaled table xts = a^2 * x applied as a
    second accumulating matmul into the dense PSUM.
  - Layer 2's table is out1, exchanged via 4 chunked AllGathers (25 blocks
    each) pipelined against edge-phase compute: chunk c is gathered by layer 2
    as soon as AG_c lands while later chunks are still being produced.
  - Layer-1 tiles run (chunk, window)-major with a bf16 SBUF accumulator so
    each gather window is one long dma_gather run (fewer SWDGE calls); per-
    window PSUM partials are ACT-evicted to bf16 and combined on the DVE in
    its all-16-bit fast mode.
  - Selection matrices S[e, d] = norm_e * (d == dst_e) are built on the DVE
    per tile from compact per-edge metadata (one fused is_equal*mult
    tensor_scalar against a constant bf16 iota tile, f32 scalars to keep the
    DVE 2x 16-bit mode) -- no S-matrix DMA from HBM.
  - Everything bf16 except PSUM accumulation (f32), the layer-2 inter-chunk
    accumulator (f32), and the epilogue math; output written bf16 and
    upcast to f32 on host.
  - Nodes are padded to 102400 and dealt round-robin over the 800
    (core, block) pairs by descending in-degree so per-block edge counts are
    balanced; all device-side structures live in position space.
"""

import numpy as np

import concourse.bass as bass
import concourse.bacc as bacc
import concourse.mybir as mybir
import concourse.tile as tile
from concourse.bass_utils import run_bass_kernel_spmd

N = 100000
E = 640000
D = 128
NCORES = 8
NPAD = 102400
SHARD = NPAD // NCORES        # 12800
NBLK = SHARD // 128           # 100 dst blocks per core
WIN = 25600                   # gather window rows (int16-safe); NPAD/WIN = 4
NWIN = NPAD // WIN            # 4
NCHUNK = 4                    # AllGather chunks for the layer-2 table
CBLK = NBLK // NCHUNK         # 25 blocks per chunk
SUB = 5                       # blocks per layer-1 PSUM sub-chunk (banks)
NSUB = CBLK // SUB            # 5 sub-chunks per chunk
CHUNK_T = 8                   # tiles (128 edges) per dma_gather call

BF = mybir.dt.bfloat16
NPBF = mybir.dt.np(BF)

_CACHE = {}
PADNEG = False   # pad gather slots with -1 (DMA skips them) instead of 0 (crashes HW; keep False)
QUEUES = 4       # SWDGE queues for gather round-robin
SCRATCH = None   # dynamic_dma_scratch_size override


def _group_schedule(gid, rel, nrm, dloc, ngroups, counts_max):
    """Common-tile-schedule packing for one layer on one core.

    Returns flat (per 128-slot tile) gidx/norm/dst arrays in tile order.
    """
    T = (counts_max + 127) // 128
    T = np.maximum(T, 1)
    tile_base = np.zeros(ngroups + 1, np.int64)
    tile_base[1:] = np.cumsum(T)
    t_total = int(tile_base[-1])

    order = np.lexsort((rel, gid))
    gid_s = gid[order]
    counts = np.bincount(gid_s, minlength=ngroups)
    grp_off = np.zeros(ngroups + 1, np.int64)
    grp_off[1:] = np.cumsum(counts)
    rank = np.arange(gid_s.shape[0], dtype=np.int64) - grp_off[gid_s]
    pos = tile_base[gid_s] * 128 + rank

    gidx = np.full(t_total * 128, -1 if PADNEG else 0, np.int16)
    norm = np.zeros(t_total * 128, np.float32)
    dst = np.zeros(t_total * 128, np.int64)
    gidx[pos] = rel[order].astype(np.int16)
    norm[pos] = nrm[order]
    dst[pos] = dloc[order]
    return T, tile_base, t_total, gidx, norm, dst


def _wrap_idx(gidx, calls, t_total):
    idxw = np.zeros((128, t_total * 8), np.int16)
    for (w, t0, nt) in calls:
        blk = gidx[t0 * 128:(t0 + nt) * 128].reshape(nt * 8, 16).T
        idxw[:, t0 * 8:(t0 + nt) * 8] = np.tile(blk, (8, 1))
    return idxw


def _calls_from_runs(tile_base, runs):
    """runs: list of (window, gid_start, gid_end). Chop each run's tile range
    into dma_gather calls of <= CHUNK_T tiles."""
    calls = []
    for (w, g0, g1) in runs:
        t = int(tile_base[g0])
        t_end = int(tile_base[g1])
        while t < t_end:
            nt = min(CHUNK_T, t_end - t)
            calls.append((w, t, nt))
            t += nt
    return calls


def _host_prep(x, edge_index, W1, b1, W2, b2):
    x = np.asarray(x, dtype=np.float32)
    ei = np.asarray(edge_index)
    W1 = np.asarray(W1, dtype=np.float32)
    W2 = np.asarray(W2, dtype=np.float32)
    b1 = np.asarray(b1, dtype=np.float32)
    b2 = np.asarray(b2, dtype=np.float32)
    n = x.shape[0]

    src = np.concatenate([ei[0].astype(np.int64), np.arange(n, dtype=np.int64)])
    dst = np.concatenate([ei[1].astype(np.int64), np.arange(n, dtype=np.int64)])
    deg = np.bincount(dst, minlength=NPAD).astype(np.float32)
    a = np.zeros(NPAD, np.float32)
    nz = deg > 0
    a[nz] = 1.0 / np.sqrt(deg[nz])

    # degree-balanced node->position permutation (nodes dealt round-robin over
    # the 800 (core, block) pairs by descending degree)
    order_by_deg = np.argsort(-deg, kind="stable")
    i = np.arange(NPAD, dtype=np.int64)
    cb = i % (NCORES * NBLK)
    position_of_rank = (cb % NCORES) * SHARD + (cb // NCORES) * 128 + i // (NCORES * NBLK)
    pos_of_node = np.empty(NPAD, np.int64)
    pos_of_node[order_by_deg] = position_of_rank
    node_at_pos = np.empty(NPAD, np.int64)
    node_at_pos[pos_of_node] = i

    ps = pos_of_node[src]
    pd = pos_of_node[dst]
    core = pd // SHARD
    norm_all = a[src] * a[dst]
    is_self = np.zeros(src.shape[0], bool)
    is_self[E:] = True          # the appended self-loops

    x_pad = np.zeros((NPAD, D), np.float32)
    x_pad[:n] = x
    x_perm = x_pad[node_at_pos]
    x_perm_bf = np.ascontiguousarray(x_perm.astype(NPBF))
    a_pos = a[node_at_pos]
    # per-position self-loop contribution table: x[d] * a_d^2, feature-major
    xts_full = (x_perm * (a_pos ** 2)[:, None]).astype(NPBF)

    NG1 = NCHUNK * NSUB * NWIN * SUB     # 400
    NG2 = NCHUNK * NBLK                  # 400

    per_core = []
    cmax1 = np.zeros(NG1, np.int64)
    cmax2 = np.zeros(NG2, np.int64)
    for k in range(NCORES):
        m = core == k
        m1 = m & ~is_self               # layer 1 skips self-loops (handled densely)
        s1k = ps[m1]
        d1k = pd[m1]
        nrm1 = norm_all[m1]
        blk1 = (d1k % SHARD) // 128
        dloc1 = d1k % 128
        c1 = blk1 // CBLK
        ss1 = (blk1 % CBLK) // SUB
        b5 = blk1 % SUB
        w1 = s1k // WIN
        gid1 = ((c1 * NWIN + w1) * NSUB + ss1) * SUB + b5
        rel1 = s1k - w1 * WIN

        s_k = ps[m]
        d_k = pd[m]
        nrm = norm_all[m]
        blk = (d_k % SHARD) // 128
        dloc = d_k % 128
        sb = (s_k % SHARD) // 128        # source block 0..99
        c2 = sb // CBLK
        rel2 = (s_k // SHARD) * (CBLK * 128) + (sb % CBLK) * 128 + (s_k % 128)
        gid2 = c2 * NBLK + blk

        cmax1 = np.maximum(cmax1, np.bincount(gid1, minlength=NG1))
        cmax2 = np.maximum(cmax2, np.bincount(gid2, minlength=NG2))
        per_core.append((gid1, rel1, nrm1, dloc1, gid2, rel2, nrm, dloc))

    T1 = (cmax1 + 127) // 128
    T1 = np.maximum(T1, 1)
    tb1 = np.zeros(NG1 + 1, np.int64)
    tb1[1:] = np.cumsum(T1)
    t1 = int(tb1[-1])
    T2 = (cmax2 + 127) // 128
    T2 = np.maximum(T2, 1)
    tb2 = np.zeros(NG2 + 1, np.int64)
    tb2[1:] = np.cumsum(T2)
    t2 = int(tb2[-1])

    # gather-call schedule (identical across cores)
    runs1 = []
    for c in range(NCHUNK):
        for w in range(NWIN):
            g0 = (c * NWIN + w) * NSUB * SUB
            runs1.append((w, g0, g0 + NSUB * SUB))
    calls1 = _calls_from_runs(tb1, runs1)
    runs2 = [(c, c * NBLK, (c + 1) * NBLK) for c in range(NCHUNK)]
    calls2 = _calls_from_runs(tb2, runs2)

    in_maps = []
    b1nz = bool(np.any(b1 != 0.0))
    b2nz = bool(np.any(b2 != 0.0))
    for k in range(NCORES):
        gid1, rel1, nrm1, dloc1, gid2, rel2, nrm, dloc = per_core[k]
        _, _, _, gidx1, norm1, dst1 = _group_schedule(
            gid1, rel1, nrm1, dloc1, NG1, cmax1)
        _, _, _, gidx2, norm2, dst2 = _group_schedule(
            gid2, rel2, nrm, dloc, NG2, cmax2)
        im = {
            "x": x_perm_bf,
            "xts": np.ascontiguousarray(
                xts_full[k * SHARD:(k + 1) * SHARD].T),
            "idx1": _wrap_idx(gidx1, calls1, t1),
            "md1": np.ascontiguousarray(dst1.reshape(t1, 128).T.astype(np.float32)),
            "mn1": np.ascontiguousarray(norm1.reshape(t1, 128).T.astype(np.float32)),
            "idx2": _wrap_idx(gidx2, calls2, t2),
            "md2": np.ascontiguousarray(dst2.reshape(t2, 128).T.astype(np.float32)),
            "mn2": np.ascontiguousarray(norm2.reshape(t2, 128).T.astype(np.float32)),
            "W1": W1.astype(NPBF),
            "W2": W2.astype(NPBF),
            "b1bc": np.broadcast_to(b1, (128, 128)).astype(np.float32).copy(),
            "b2bc": np.broadcast_to(b2, (128, 128)).astype(np.float32).copy(),
        }
        in_maps.append(im)

    sched = {
        "T1": tuple(int(v) for v in T1),
        "T2": tuple(int(v) for v in T2),
        "calls1": tuple(calls1),
        "calls2": tuple(calls2),
        "t1": t1,
        "t2": t2,
        "b1nz": b1nz,
        "b2nz": b2nz,
    }
    return in_maps, sched, pos_of_node


def _build_program(sched, variant="full"):
    T1 = np.array(sched["T1"], np.int64)
    T2 = np.array(sched["T2"], np.int64)
    tb1 = np.zeros(T1.shape[0] + 1, np.int64)
    tb1[1:] = np.cumsum(T1)
    tb2 = np.zeros(T2.shape[0] + 1, np.int64)
    tb2[1:] = np.cumsum(T2)
    calls1 = list(sched["calls1"])
    calls2 = list(sched["calls2"])
    t1, t2 = sched["t1"], sched["t2"]
    b1nz, b2nz = sched["b1nz"], sched["b2nz"]

    # per-tile annotations, layer 1: block id + first/last flags
    # gid1 -> (c, s, w, b5); block = (c*NSUB + s)*SUB + b5
    blk_of_t1 = np.zeros(t1, np.int64)
    w_of_t1 = np.zeros(t1, np.int64)
    first_t1 = np.zeros(t1, bool)
    last_t1 = np.zeros(t1, bool)
    for g in range(T1.shape[0]):
        b5 = g % SUB
        s = (g // SUB) % NSUB
        w = (g // (SUB * NSUB)) % NWIN
        c = g // (SUB * NSUB * NWIN)
        b = (c * NSUB + s) * SUB + b5
        blk_of_t1[tb1[g]:tb1[g + 1]] = b
        w_of_t1[tb1[g]:tb1[g + 1]] = w
        first_t1[tb1[g]] = True
        last_t1[tb1[g + 1] - 1] = True
    # layer 2: gid2 = c*NBLK + b
    blk_of_t2 = np.zeros(t2, np.int64)
    c_of_t2 = np.zeros(t2, np.int64)
    first_t2 = np.zeros(t2, bool)
    last_t2 = np.zeros(t2, bool)
    for g in range(T2.shape[0]):
        c = g // NBLK
        b = g % NBLK
        blk_of_t2[tb2[g]:tb2[g + 1]] = b
        c_of_t2[tb2[g]:tb2[g + 1]] = c
        first_t2[tb2[g]] = True
        last_t2[tb2[g + 1] - 1] = True

    kw = {}
    if SCRATCH is not None:
        kw["dynamic_dma_scratch_size"] = SCRATCH
    nc = bacc.Bacc("TRN2", target_bir_lowering=False, debug=False,
                   num_devices=NCORES, num_swdge_queues=QUEUES, **kw)
    f32 = mybir.dt.float32
    i16 = mybir.dt.int16

    x_d = nc.dram_tensor("x", [NPAD, D], BF, kind="ExternalInput")
    xts_d = nc.dram_tensor("xts", [D, SHARD], BF, kind="ExternalInput")
    idx1_d = nc.dram_tensor("idx1", [128, t1 * 8], i16, kind="ExternalInput")
    md1_d = nc.dram_tensor("md1", [128, t1], f32, kind="ExternalInput")
    mn1_d = nc.dram_tensor("mn1", [128, t1], f32, kind="ExternalInput")
    idx2_d = nc.dram_tensor("idx2", [128, t2 * 8], i16, kind="ExternalInput")
    md2_d = nc.dram_tensor("md2", [128, t2], f32, kind="ExternalInput")
    mn2_d = nc.dram_tensor("mn2", [128, t2], f32, kind="ExternalInput")
    W1_d = nc.dram_tensor("W1", [D, D], BF, kind="ExternalInput")
    W2_d = nc.dram_tensor("W2", [D, D], BF, kind="ExternalInput")
    b1_d = nc.dram_tensor("b1bc", [128, 128], f32, kind="ExternalInput")
    b2_d = nc.dram_tensor("b2bc", [128, 128], f32, kind="ExternalInput")
    out_d = nc.dram_tensor("out", [SHARD, D], BF, kind="ExternalOutput")

    ob = [nc.dram_tensor(f"ob{c}", [CBLK * 128, D], BF) for c in range(NCHUNK)]
    hf = [nc.dram_tensor(f"hf{c}", [CBLK * 128 * NCORES, D], BF,
                         addr_space="Shared") for c in range(NCHUNK)]

    with tile.TileContext(nc) as tc:
        with (
            tc.tile_pool(name="const", bufs=1) as p_const,
            tc.tile_pool(name="acc", bufs=1) as p_acc,
            tc.tile_pool(name="msg", bufs=8) as p_msg,
            tc.tile_pool(name="sel", bufs=8) as p_sel,
            tc.tile_pool(name="agg", bufs=4) as p_agg,
            tc.tile_pool(name="o1", bufs=4) as p_o1,
            tc.tile_pool(name="tmp", bufs=2) as p_tmp,
            tc.tile_pool(name="eps", bufs=SUB, space="PSUM") as p_eps,
            tc.tile_pool(name="dps", bufs=2, space="PSUM") as p_dps,
        ):
            W1_t = p_const.tile([D, D], BF)
            W2_t = p_const.tile([D, D], BF)
            idx1_t = p_const.tile([128, t1 * 8], i16)
            md1_t = p_const.tile([128, t1], f32)
            mn1_t = p_const.tile([128, t1], f32)
            idx2_t = p_const.tile([128, t2 * 8], i16)
            md2_t = p_const.tile([128, t2], f32)
            mn2_t = p_const.tile([128, t2], f32)
            nc.sync.dma_start(out=idx1_t[:], in_=idx1_d[:])
            nc.sync.dma_start(out=md1_t[:], in_=md1_d[:])
            nc.sync.dma_start(out=mn1_t[:], in_=mn1_d[:])
            nc.sync.dma_start(out=W1_t[:], in_=W1_d[:])
            nc.sync.dma_start(out=W2_t[:], in_=W2_d[:])
            nc.sync.dma_start(out=idx2_t[:], in_=idx2_d[:])
            nc.sync.dma_start(out=md2_t[:], in_=md2_d[:])
            nc.sync.dma_start(out=mn2_t[:], in_=mn2_d[:])
            if b1nz or b2nz:
                b1_t = p_const.tile([128, 128], f32)
                b2_t = p_const.tile([128, 128], f32)
                nc.sync.dma_start(out=b1_t[:], in_=b1_d[:])
                nc.sync.dma_start(out=b2_t[:], in_=b2_d[:])

            xts_t = p_const.tile([D, SHARD], BF)
            nc.sync.dma_start(out=xts_t[:], in_=xts_d[:])
            iota_i = p_const.tile([128, 128], mybir.dt.int32)
            iota_bf = p_const.tile([128, 128], BF)
            nc.gpsimd.iota(out=iota_i[:], pattern=[[1, 128]], base=0,
                           channel_multiplier=0)
            nc.vector.tensor_copy(out=iota_bf[:], in_=iota_i[:])

            acc1 = p_acc.tile([128, SHARD], BF, name="acc1")
            acc2 = p_acc.tile([128, SHARD], BF)

            if PADNEG:
                for _z in range(8):
                    zt = p_msg.tile([128, CHUNK_T, D], BF, tag="msg", name="zmsg")
                    nc.vector.memset(zt[:], 0.0)

            def build_S(md_t, mn_t, t):
                S_t = p_sel.tile([128, 128], BF, tag="sel")
                nc.vector.tensor_scalar(
                    out=S_t[:], in0=iota_bf[:],
                    scalar1=md_t[:, t:t + 1], scalar2=mn_t[:, t:t + 1],
                    op0=mybir.AluOpType.is_equal, op1=mybir.AluOpType.mult)
                return S_t

            # ---------------- layer 1 ----------------
            pending = {}
            stage1 = [None]
            stage2 = [None]
            ag_ready = []
            chunk_done_emitted = [False] * NCHUNK
            def emit_ags():
                while ag_ready:
                    cr = ag_ready.pop(0)
                    nc.gpsimd.collective_compute(
                        "AllGather", mybir.AluOpType.bypass,
                        replica_groups=[list(range(NCORES))],
                        ins=[ob[cr][:]], outs=[hf[cr][:]],
                    )

            last_call_chunk = [-1, 0]
            for ci, (w, t0, nt) in enumerate(calls1):
                call_chunk = int(blk_of_t1[t0]) // CBLK
                if call_chunk != last_call_chunk[0]:
                    last_call_chunk[0] = call_chunk
                    last_call_chunk[1] = 0
                else:
                    last_call_chunk[1] += 1
                    if last_call_chunk[1] == 4:
                        emit_ags()
                msg_t = p_msg.tile([128, CHUNK_T, D], BF, tag="msg")
                nc.gpsimd.dma_gather(
                    out_ap=msg_t[:, :nt, :],
                    in_ap=x_d[w * WIN:(w + 1) * WIN, :],
                    idxs_ap=idx1_t[:, t0 * 8:(t0 + nt) * 8],
                    num_idxs=nt * 128, num_idxs_reg=nt * 128,
                    elem_size=D, queue_num=ci % QUEUES)
                for t in range(t0, t0 + nt):
                    b = int(blk_of_t1[t])
                    w_t = int(w_of_t1[t])
                    S_t = build_S(md1_t, mn1_t, t)
                    if first_t1[t]:
                        pending[b] = p_eps.tile([128, D], f32, space="PSUM",
                                                tag="eps", name="eps")
                    ps = pending[b]
                    nc.tensor.matmul(out=ps[:], lhsT=msg_t[:, t - t0, :],
                                     rhs=S_t[:], start=bool(first_t1[t]),
                                     stop=bool(last_t1[t]))
                    if last_t1[t] and w_t < NWIN - 1:
                        # evict this window's partial into the bf16 accumulator
                        del pending[b]
                        a1blk = acc1[:, b * 128:(b + 1) * 128]
                        if w_t == 0:
                            nc.scalar.activation(
                                out=a1blk, in_=ps[:],
                                func=mybir.ActivationFunctionType.Copy)
                        else:
                            tb_ = p_agg.tile([128, D], BF, tag="agg",
                                             name="tb1")
                            nc.scalar.activation(
                                out=tb_[:], in_=ps[:],
                                func=mybir.ActivationFunctionType.Copy)
                            nc.vector.tensor_add(out=a1blk, in0=a1blk,
                                                 in1=tb_[:])
                    elif last_t1[t]:
                        del pending[b]
                        tb_ = p_agg.tile([128, D], BF, tag="agg", name="tb1")
                        nc.scalar.activation(
                            out=tb_[:], in_=ps[:],
                            func=mybir.ActivationFunctionType.Copy)
                        aggbf = p_agg.tile([128, D], BF, tag="agg")
                        nc.vector.tensor_add(out=aggbf[:],
                                             in0=acc1[:, b * 128:(b + 1) * 128],
                                             in1=tb_[:])
                        ps2 = p_dps.tile([128, D], f32, space="PSUM", tag="dps")
                        nc.tensor.matmul(out=ps2[:], lhsT=aggbf[:], rhs=W1_t[:],
                                         start=True, stop=False)
                        nc.tensor.matmul(
                            out=ps2[:], lhsT=xts_t[:, b * 128:(b + 1) * 128],
                            rhs=W1_t[:], start=False, stop=True)
                        b5 = b % SUB
                        if b5 == 0:
                            stage1[0] = p_o1.tile([128, SUB * D], BF,
                                                  tag="o1", name="o1w")
                        o1 = stage1[0][:, b5 * D:(b5 + 1) * D]
                        if b1nz:
                            tmp = p_tmp.tile([128, D], f32, tag="tmp")
                            nc.vector.tensor_add(out=tmp[:], in0=ps2[:],
                                                 in1=b1_t[:])
                            nc.scalar.activation(
                                out=o1, in_=tmp[:],
                                func=mybir.ActivationFunctionType.Relu)
                        else:
                            nc.scalar.activation(
                                out=o1, in_=ps2[:],
                                func=mybir.ActivationFunctionType.Relu)
                        c = b // CBLK
                        if b5 == SUB - 1:
                            sl = (b % CBLK) // SUB
                            nc.sync.dma_start(
                                out=ob[c][sl * SUB * 128:(sl + 1) * SUB * 128,
                                          :].rearrange("(j r) f -> r j f",
                                                       j=SUB),
                                in_=stage1[0][:])
                        # mark chunk c ready; the AllGather is emitted after
                        # the NEXT chunk's gather calls so the Pool queue
                        # keeps generating descriptors while AG_c waits on
                        # the bounce writes
                        if (b % CBLK) == CBLK - 1 and variant == "full":
                            assert not chunk_done_emitted[c]
                            chunk_done_emitted[c] = True
                            ag_ready.append(c)

            if variant == "full":
                emit_ags()

            # ---------------- layer 2 ----------------
            pending2 = {}
            for ci, (c, t0, nt) in enumerate(calls2):
                msg_t = p_msg.tile([128, CHUNK_T, D], BF, tag="msg")
                src_ap = hf[c][:] if variant == "full" else x_d[0:CBLK * 128 * NCORES, :]
                nc.gpsimd.dma_gather(
                    out_ap=msg_t[:, :nt, :],
                    in_ap=src_ap,
                    idxs_ap=idx2_t[:, t0 * 8:(t0 + nt) * 8],
                    num_idxs=nt * 128, num_idxs_reg=nt * 128,
                    elem_size=D, queue_num=ci % QUEUES)
                for t in range(t0, t0 + nt):
                    b = int(blk_of_t2[t])
                    S_t = build_S(md2_t, mn2_t, t)
                    if first_t2[t]:
                        pending2[b] = p_eps.tile([128, D], f32, space="PSUM",
                                                 tag="eps", name="eps")
                    ps = pending2[b]
                    nc.tensor.matmul(out=ps[:], lhsT=msg_t[:, t - t0, :],
                                     rhs=S_t[:], start=bool(first_t2[t]),
                                     stop=bool(last_t2[t]))
                    if last_t2[t]:
                        del pending2[b]
                        accblk = acc2[:, b * 128:(b + 1) * 128]
                        if c == 0:
                            nc.scalar.activation(
                                out=accblk, in_=ps[:],
                                func=mybir.ActivationFunctionType.Copy)
                        elif c < NCHUNK - 1:
                            # ACT evicts PSUM to bf16 so the DVE add runs in
                            # the all-16-bit fast mode
                            tbf = p_agg.tile([128, D], BF, tag="agg",
                                             name="tbf")
                            nc.scalar.activation(
                                out=tbf[:], in_=ps[:],
                                func=mybir.ActivationFunctionType.Copy)
                            nc.vector.tensor_add(out=accblk, in0=accblk,
                                                 in1=tbf[:])
                        else:
                            tbf = p_agg.tile([128, D], BF, tag="agg",
                                             name="tbf")
                            nc.scalar.activation(
                                out=tbf[:], in_=ps[:],
                                func=mybir.ActivationFunctionType.Copy)
                            agg2 = p_agg.tile([128, D], BF, tag="agg")
                            nc.vector.tensor_add(out=agg2[:], in0=accblk,
                                                 in1=tbf[:])
                            ps2 = p_dps.tile([128, D], f32, space="PSUM",
                                             tag="dps")
                            nc.tensor.matmul(out=ps2[:], lhsT=agg2[:],
                                             rhs=W2_t[:], start=True, stop=True)
                            b5 = b % SUB
                            if b5 == 0:
                                stage2[0] = p_o1.tile([128, SUB * D], BF,
                                                      tag="o1", name="otw")
                            ot = stage2[0][:, b5 * D:(b5 + 1) * D]
                            if b2nz:
                                tmp = p_tmp.tile([128, D], f32, tag="tmp")
                                nc.vector.tensor_add(out=tmp[:], in0=ps2[:],
                                                     in1=b2_t[:])
                                nc.scalar.activation(
                                    out=ot, in_=tmp[:],
                                    func=mybir.ActivationFunctionType.Relu)
                            else:
                                nc.scalar.activation(
                                    out=ot, in_=ps2[:],
                                    func=mybir.ActivationFunctionType.Relu)
                            if b5 == SUB - 1:
                                nc.sync.dma_start(
                                    out=out_d[(b - SUB + 1) * 128:
                                              (b + 1) * 128, :].rearrange(
                                        "(j r) f -> r j f", j=SUB),
                                    in_=stage2[0][:])

    nc.compile()
    return nc


def prepare(x, edge_index, W1, b1, W2, b2, variant="full"):
    in_maps, sched, pos_of_node = _host_prep(x, edge_index, W1, b1, W2, b2)
    key = (sched["T1"], sched["T2"], sched["calls1"], sched["calls2"],
           sched["b1nz"], sched["b2nz"], variant, PADNEG, QUEUES, SCRATCH)
    if key not in _CACHE:
        _CACHE[key] = _build_program(sched, variant)
    return _CACHE[key], in_maps, pos_of_node


def kernel(x, edge_index, W1, b1, W2, b2):
    nc, in_maps, pos_of_node = prepare(x, edge_index, W1, b1, W2, b2)
    res = run_bass_kernel_spmd(nc, in_maps, list(range(NCORES)))
    full = np.concatenate([res.results[k]["out"] for k in range(NCORES)], axis=0)
    n = np.asarray(x).shape[0]
    return np.ascontiguousarray(full[pos_of_node[:n]].astype(np.float32))


# revision 13
# speedup vs baseline: 3.2170x; 1.6205x over previous
"""Two-layer GCN (PyG GCNConv x2 + ReLU) on 8 Trainium2 NeuronCores.

Strategy (dst-sharded SPMD, aggregation-first):
  - GCN layer = relu((A_hat @ x) @ W + b): the dense matmul commutes with the
    aggregation, so each layer gathers rows of the (bf16) node table, scatter-
    adds them via on-device-built selection-matrix matmuls into per-block
    PSUM, then applies the 128x128 dense + ReLU per 128-node block.
  - Layer 1's table is x itself (replicated input) -> no dense pre-pass and no
    collective before layer 1. Self-loop contributions bypass the gather
    entirely: they are a host-prescaled table xts = a^2 * x applied as a
    second accumulating matmul into the dense PSUM.
  - Layer 2's table is out1, exchanged via 4 chunked AllGathers (25 blocks
    each) pipelined against edge-phase compute: chunk c is gathered by layer 2
    as soon as AG_c lands while later chunks are still being produced.
  - Layer-1 tiles run (chunk, window)-major with a bf16 SBUF accumulator so
    each gather window is one long dma_gather run (fewer SWDGE calls); per-
    window PSUM partials are ACT-evicted to bf16 and combined on the DVE in
    its all-16-bit fast mode.
  - Selection matrices S[e, d] = norm_e * (d == dst_e) are built on the DVE
    per tile from compact per-edge metadata (one fused is_equal*mult
    tensor_scalar against a constant bf16 iota tile, f32 scalars to keep the
    DVE 2x 16-bit mode) -- no S-matrix DMA from HBM.
  - Everything bf16 except PSUM accumulation (f32), the layer-2 inter-chunk
    accumulator (f32), and the epilogue math; output written bf16 and
    upcast to f32 on host.
  - Nodes are padded to 102400 and dealt round-robin over the 800
    (core, block) pairs by descending in-degree so per-block edge counts are
    balanced; all device-side structures live in position space.
"""

import numpy as np

import concourse.bass as bass
import concourse.bacc as bacc
import concourse.mybir as mybir
import concourse.tile as tile
from concourse.bass_utils import run_bass_kernel_spmd

N = 100000
E = 640000
D = 128
NCORES = 8
NPAD = 102400
SHARD = NPAD // NCORES        # 12800
NBLK = SHARD // 128           # 100 dst blocks per core
WIN = 25600                   # gather window rows (int16-safe); NPAD/WIN = 4
NWIN = NPAD // WIN            # 4
NCHUNK = 4                    # AllGather chunks for the layer-2 table
CBLK = NBLK // NCHUNK         # 25 blocks per chunk
SUB = 5                       # blocks per layer-1 PSUM sub-chunk (banks)
NSUB = CBLK // SUB            # 5 sub-chunks per chunk
CHUNK_T = 8                   # tiles (128 edges) per dma_gather call

BF = mybir.dt.bfloat16
NPBF = mybir.dt.np(BF)

_CACHE = {}
PADNEG = False   # pad gather slots with -1 (DMA skips them) instead of 0 (crashes HW; keep False)
QUEUES = 4       # SWDGE queues for gather round-robin
SCRATCH = None   # dynamic_dma_scratch_size override


def _group_schedule(gid, rel, nrm, dloc, ngroups, counts_max):
    """Common-tile-schedule packing for one layer on one core.

    Returns flat (per 128-slot tile) gidx/norm/dst arrays in tile order.
    """
    T = (counts_max + 127) // 128
    T = np.maximum(T, 1)
    tile_base = np.zeros(ngroups + 1, np.int64)
    tile_base[1:] = np.cumsum(T)
    t_total = int(tile_base[-1])

    order = np.lexsort((rel, gid))
    gid_s = gid[order]
    counts = np.bincount(gid_s, minlength=ngroups)
    grp_off = np.zeros(ngroups + 1, np.int64)
    grp_off[1:] = np.cumsum(counts)
    rank = np.arange(gid_s.shape[0], dtype=np.int64) - grp_off[gid_s]
    pos = tile_base[gid_s] * 128 + rank

    gidx = np.full(t_total * 128, -1 if PADNEG else 0, np.int16)
    norm = np.zeros(t_total * 128, np.float32)
    dst = np.zeros(t_total * 128, np.int64)
    gidx[pos] = rel[order].astype(np.int16)
    norm[pos] = nrm[order]
    dst[pos] = dloc[order]
    return T, tile_base, t_total, gidx, norm, dst


def _wrap_idx(gidx, calls, t_total):
    idxw = np.zeros((128, t_total * 8), np.int16)
    for (w, t0, nt) in calls:
        blk = gidx[t0 * 128:(t0 + nt) * 128].reshape(nt * 8, 16).T
        idxw[:, t0 * 8:(t0 + nt) * 8] = np.tile(blk, (8, 1))
    return idxw


def _calls_from_runs(tile_base, runs):
    """runs: list of (window, gid_start, gid_end). Chop each run's tile range
    into dma_gather calls of <= CHUNK_T tiles."""
    calls = []
    for (w, g0, g1) in runs:
        t = int(tile_base[g0])
        t_end = int(tile_base[g1])
        while t < t_end:
            nt = min(CHUNK_T, t_end - t)
            calls.append((w, t, nt))
            t += nt
    return calls


def _host_prep(x, edge_index, W1, b1, W2, b2):
    x = np.asarray(x, dtype=np.float32)
    ei = np.asarray(edge_index)
    W1 = np.asarray(W1, dtype=np.float32)
    W2 = np.asarray(W2, dtype=np.float32)
    b1 = np.asarray(b1, dtype=np.float32)
    b2 = np.asarray(b2, dtype=np.float32)
    n = x.shape[0]

    src = np.concatenate([ei[0].astype(np.int64), np.arange(n, dtype=np.int64)])
    dst = np.concatenate([ei[1].astype(np.int64), np.arange(n, dtype=np.int64)])
    deg = np.bincount(dst, minlength=NPAD).astype(np.float32)
    a = np.zeros(NPAD, np.float32)
    nz = deg > 0
    a[nz] = 1.0 / np.sqrt(deg[nz])

    # degree-balanced node->position permutation (nodes dealt round-robin over
    # the 800 (core, block) pairs by descending degree)
    order_by_deg = np.argsort(-deg, kind="stable")
    i = np.arange(NPAD, dtype=np.int64)
    cb = i % (NCORES * NBLK)
    position_of_rank = (cb % NCORES) * SHARD + (cb // NCORES) * 128 + i // (NCORES * NBLK)
    pos_of_node = np.empty(NPAD, np.int64)
    pos_of_node[order_by_deg] = position_of_rank
    node_at_pos = np.empty(NPAD, np.int64)
    node_at_pos[pos_of_node] = i

    ps = pos_of_node[src]
    pd = pos_of_node[dst]
    core = pd // SHARD
    norm_all = a[src] * a[dst]
    is_self = np.zeros(src.shape[0], bool)
    is_self[E:] = True          # the appended self-loops

    x_pad = np.zeros((NPAD, D), np.float32)
    x_pad[:n] = x
    x_perm = x_pad[node_at_pos]
    x_perm_bf = np.ascontiguousarray(x_perm.astype(NPBF))
    a_pos = a[node_at_pos]
    # per-position self-loop contribution table: x[d] * a_d^2, feature-major
    xts_full = (x_perm * (a_pos ** 2)[:, None]).astype(NPBF)

    NG1 = NCHUNK * NSUB * NWIN * SUB     # 400
    NG2 = NCHUNK * NBLK                  # 400

    per_core = []
    cmax1 = np.zeros(NG1, np.int64)
    cmax2 = np.zeros(NG2, np.int64)
    for k in range(NCORES):
        m = core == k
        m1 = m & ~is_self               # layer 1 skips self-loops (handled densely)
        s1k = ps[m1]
        d1k = pd[m1]
        nrm1 = norm_all[m1]
        blk1 = (d1k % SHARD) // 128
        dloc1 = d1k % 128
        c1 = blk1 // CBLK
        ss1 = (blk1 % CBLK) // SUB
        b5 = blk1 % SUB
        w1 = s1k // WIN
        gid1 = ((c1 * NWIN + w1) * NSUB + ss1) * SUB + b5
        rel1 = s1k - w1 * WIN

        s_k = ps[m]
        d_k = pd[m]
        nrm = norm_all[m]
        blk = (d_k % SHARD) // 128
        dloc = d_k % 128
        sb = (s_k % SHARD) // 128        # source block 0..99
        c2 = sb // CBLK
        rel2 = (s_k // SHARD) * (CBLK * 128) + (sb % CBLK) * 128 + (s_k % 128)
        gid2 = c2 * NBLK + blk

        cmax1 = np.maximum(cmax1, np.bincount(gid1, minlength=NG1))
        cmax2 = np.maximum(cmax2, np.bincount(gid2, minlength=NG2))
        per_core.append((gid1, rel1, nrm1, dloc1, gid2, rel2, nrm, dloc))

    T1 = (cmax1 + 127) // 128
    T1 = np.maximum(T1, 1)
    tb1 = np.zeros(NG1 + 1, np.int64)
    tb1[1:] = np.cumsum(T1)
    t1 = int(tb1[-1])
    T2 = (cmax2 + 127) // 128
    T2 = np.maximum(T2, 1)
    tb2 = np.zeros(NG2 + 1, np.int64)
    tb2[1:] = np.cumsum(T2)
    t2 = int(tb2[-1])

    # gather-call schedule (identical across cores)
    runs1 = []
    for c in range(NCHUNK):
        for w in range(NWIN):
            g0 = (c * NWIN + w) * NSUB * SUB
            runs1.append((w, g0, g0 + NSUB * SUB))
    calls1 = _calls_from_runs(tb1, runs1)
    runs2 = [(c, c * NBLK, (c + 1) * NBLK) for c in range(NCHUNK)]
    calls2 = _calls_from_runs(tb2, runs2)

    in_maps = []
    b1nz = bool(np.any(b1 != 0.0))
    b2nz = bool(np.any(b2 != 0.0))
    for k in range(NCORES):
        gid1, rel1, nrm1, dloc1, gid2, rel2, nrm, dloc = per_core[k]
        _, _, _, gidx1, norm1, dst1 = _group_schedule(
            gid1, rel1, nrm1, dloc1, NG1, cmax1)
        _, _, _, gidx2, norm2, dst2 = _group_schedule(
            gid2, rel2, nrm, dloc, NG2, cmax2)
        im = {
            "x": x_perm_bf,
            "xts": np.ascontiguousarray(
                xts_full[k * SHARD:(k + 1) * SHARD].T),
            "idx1": _wrap_idx(gidx1, calls1, t1),
            "md1": np.ascontiguousarray(dst1.reshape(t1, 128).T.astype(np.float32)),
            "mn1": np.ascontiguousarray(norm1.reshape(t1, 128).T.astype(np.float32)),
            "idx2": _wrap_idx(gidx2, calls2, t2),
            "md2": np.ascontiguousarray(dst2.reshape(t2, 128).T.astype(np.float32)),
            "mn2": np.ascontiguousarray(norm2.reshape(t2, 128).T.astype(np.float32)),
            "W1": W1.astype(NPBF),
            "W2": W2.astype(NPBF),
            "b1bc": np.broadcast_to(b1, (128, 128)).astype(np.float32).copy(),
            "b2bc": np.broadcast_to(b2, (128, 128)).astype(np.float32).copy(),
        }
        in_maps.append(im)

    sched = {
        "T1": tuple(int(v) for v in T1),
        "T2": tuple(int(v) for v in T2),
        "calls1": tuple(calls1),
        "calls2": tuple(calls2),
        "t1": t1,
        "t2": t2,
        "b1nz": b1nz,
        "b2nz": b2nz,
    }
    return in_maps, sched, pos_of_node


def _build_program(sched, variant="full"):
    T1 = np.array(sched["T1"], np.int64)
    T2 = np.array(sched["T2"], np.int64)
    tb1 = np.zeros(T1.shape[0] + 1, np.int64)
    tb1[1:] = np.cumsum(T1)
    tb2 = np.zeros(T2.shape[0] + 1, np.int64)
    tb2[1:] = np.cumsum(T2)
    calls1 = list(sched["calls1"])
    calls2 = list(sched["calls2"])
    t1, t2 = sched["t1"], sched["t2"]
    b1nz, b2nz = sched["b1nz"], sched["b2nz"]

    # per-tile annotations, layer 1: block id + first/last flags
    # gid1 -> (c, s, w, b5); block = (c*NSUB + s)*SUB + b5
    blk_of_t1 = np.zeros(t1, np.int64)
    w_of_t1 = np.zeros(t1, np.int64)
    first_t1 = np.zeros(t1, bool)
    last_t1 = np.zeros(t1, bool)
    for g in range(T1.shape[0]):
        b5 = g % SUB
        s = (g // SUB) % NSUB
        w = (g // (SUB * NSUB)) % NWIN
        c = g // (SUB * NSUB * NWIN)
        b = (c * NSUB + s) * SUB + b5
        blk_of_t1[tb1[g]:tb1[g + 1]] = b
        w_of_t1[tb1[g]:tb1[g + 1]] = w
        first_t1[tb1[g]] = True
        last_t1[tb1[g + 1] - 1] = True
    # layer 2: gid2 = c*NBLK + b
    blk_of_t2 = np.zeros(t2, np.int64)
    c_of_t2 = np.zeros(t2, np.int64)
    first_t2 = np.zeros(t2, bool)
    last_t2 = np.zeros(t2, bool)
    for g in range(T2.shape[0]):
        c = g // NBLK
        b = g % NBLK
        blk_of_t2[tb2[g]:tb2[g + 1]] = b
        c_of_t2[tb2[g]:tb2[g + 1]] = c
        first_t2[tb2[g]] = True
        last_t2[tb2[g + 1] - 1] = True

    kw = {}
    if SCRATCH is not None:
        kw["dynamic_dma_scratch_size"] = SCRATCH
    nc = bacc.Bacc("TRN2", target_bir_lowering=False, debug=False,
                   num_devices=NCORES, num_swdge_queues=QUEUES, **kw)
    f32 = mybir.dt.float32
    i16 = mybir.dt.int16

    x_d = nc.dram_tensor("x", [NPAD, D], BF, kind="ExternalInput")
    xts_d = nc.dram_tensor("xts", [D, SHARD], BF, kind="ExternalInput")
    idx1_d = nc.dram_tensor("idx1", [128, t1 * 8], i16, kind="ExternalInput")
    md1_d = nc.dram_tensor("md1", [128, t1], f32, kind="ExternalInput")
    mn1_d = nc.dram_tensor("mn1", [128, t1], f32, kind="ExternalInput")
    idx2_d = nc.dram_tensor("idx2", [128, t2 * 8], i16, kind="ExternalInput")
    md2_d = nc.dram_tensor("md2", [128, t2], f32, kind="ExternalInput")
    mn2_d = nc.dram_tensor("mn2", [128, t2], f32, kind="ExternalInput")
    W1_d = nc.dram_tensor("W1", [D, D], BF, kind="ExternalInput")
    W2_d = nc.dram_tensor("W2", [D, D], BF, kind="ExternalInput")
    b1_d = nc.dram_tensor("b1bc", [128, 128], f32, kind="ExternalInput")
    b2_d = nc.dram_tensor("b2bc", [128, 128], f32, kind="ExternalInput")
    out_d = nc.dram_tensor("out", [SHARD, D], BF, kind="ExternalOutput")

    ob = [nc.dram_tensor(f"ob{c}", [CBLK * 128, D], BF) for c in range(NCHUNK)]
    hf = [nc.dram_tensor(f"hf{c}", [CBLK * 128 * NCORES, D], BF,
                         addr_space="Shared") for c in range(NCHUNK)]

    with tile.TileContext(nc) as tc:
        with (
            tc.tile_pool(name="const", bufs=1) as p_const,
            tc.tile_pool(name="acc", bufs=1) as p_acc,
            tc.tile_pool(name="msg", bufs=8) as p_msg,
            tc.tile_pool(name="sel", bufs=8) as p_sel,
            tc.tile_pool(name="agg", bufs=4) as p_agg,
            tc.tile_pool(name="o1", bufs=4) as p_o1,
            tc.tile_pool(name="tmp", bufs=2) as p_tmp,
            tc.tile_pool(name="eps", bufs=SUB, space="PSUM") as p_eps,
            tc.tile_pool(name="dps", bufs=3, space="PSUM") as p_dps,
        ):
            W1_t = p_const.tile([D, D], BF)
            W2_t = p_const.tile([D, D], BF)
            idx1_t = p_const.tile([128, t1 * 8], i16)
            md1_t = p_const.tile([128, t1], f32)
            mn1_t = p_const.tile([128, t1], f32)
            idx2_t = p_const.tile([128, t2 * 8], i16)
            md2_t = p_const.tile([128, t2], f32)
            mn2_t = p_const.tile([128, t2], f32)
            nc.sync.dma_start(out=idx1_t[:], in_=idx1_d[:])
            nc.sync.dma_start(out=md1_t[:], in_=md1_d[:])
            nc.sync.dma_start(out=mn1_t[:], in_=mn1_d[:])
            nc.sync.dma_start(out=W1_t[:], in_=W1_d[:])
            nc.sync.dma_start(out=W2_t[:], in_=W2_d[:])
            nc.sync.dma_start(out=idx2_t[:], in_=idx2_d[:])
            nc.sync.dma_start(out=md2_t[:], in_=md2_d[:])
            nc.sync.dma_start(out=mn2_t[:], in_=mn2_d[:])
            if b1nz or b2nz:
                b1_t = p_const.tile([128, 128], f32)
                b2_t = p_const.tile([128, 128], f32)
                nc.sync.dma_start(out=b1_t[:], in_=b1_d[:])
                nc.sync.dma_start(out=b2_t[:], in_=b2_d[:])

            xts_t = p_const.tile([D, SHARD], BF)
            nc.sync.dma_start(out=xts_t[:], in_=xts_d[:])
            iota_i = p_const.tile([128, 128], mybir.dt.int32)
            iota_bf = p_const.tile([128, 128], BF)
            nc.gpsimd.iota(out=iota_i[:], pattern=[[1, 128]], base=0,
                           channel_multiplier=0)
            nc.vector.tensor_copy(out=iota_bf[:], in_=iota_i[:])

            acc1 = p_acc.tile([128, SHARD], BF, name="acc1")
            acc2 = p_acc.tile([128, SHARD], BF)

            if PADNEG:
                for _z in range(8):
                    zt = p_msg.tile([128, CHUNK_T, D], BF, tag="msg", name="zmsg")
                    nc.vector.memset(zt[:], 0.0)

            def build_S(md_t, mn_t, t):
                S_t = p_sel.tile([128, 128], BF, tag="sel")
                nc.vector.tensor_scalar(
                    out=S_t[:], in0=iota_bf[:],
                    scalar1=md_t[:, t:t + 1], scalar2=mn_t[:, t:t + 1],
                    op0=mybir.AluOpType.is_equal, op1=mybir.AluOpType.mult)
                return S_t

            # ---------------- layer 1 ----------------
            pending = {}
            stage1 = [None]
            stage2 = [None]
            ag_ready = []
            chunk_done_emitted = [False] * NCHUNK
            def emit_ags():
                while ag_ready:
                    cr = ag_ready.pop(0)
                    nc.gpsimd.collective_compute(
                        "AllGather", mybir.AluOpType.bypass,
                        replica_groups=[list(range(NCORES))],
                        ins=[ob[cr][:]], outs=[hf[cr][:]],
                    )

            last_call_chunk = [-1, 0]
            for ci, (w, t0, nt) in enumerate(calls1):
                call_chunk = int(blk_of_t1[t0]) // CBLK
                if call_chunk != last_call_chunk[0]:
                    last_call_chunk[0] = call_chunk
                    last_call_chunk[1] = 0
                else:
                    last_call_chunk[1] += 1
                    if last_call_chunk[1] == 4:
                        emit_ags()
                msg_t = p_msg.tile([128, CHUNK_T, D], BF, tag="msg")
                nc.gpsimd.dma_gather(
                    out_ap=msg_t[:, :nt, :],
                    in_ap=x_d[w * WIN:(w + 1) * WIN, :],
                    idxs_ap=idx1_t[:, t0 * 8:(t0 + nt) * 8],
                    num_idxs=nt * 128, num_idxs_reg=nt * 128,
                    elem_size=D, queue_num=ci % QUEUES)
                for t in range(t0, t0 + nt):
                    b = int(blk_of_t1[t])
                    w_t = int(w_of_t1[t])
                    S_t = build_S(md1_t, mn1_t, t)
                    if first_t1[t]:
                        pending[b] = p_eps.tile([128, D], f32, space="PSUM",
                                                tag="eps", name="eps")
                    ps = pending[b]
                    nc.tensor.matmul(out=ps[:], lhsT=msg_t[:, t - t0, :],
                                     rhs=S_t[:], start=bool(first_t1[t]),
                                     stop=bool(last_t1[t]))
                    if last_t1[t] and w_t < NWIN - 1:
                        # evict this window's partial into the bf16 accumulator
                        del pending[b]
                        a1blk = acc1[:, b * 128:(b + 1) * 128]
                        if w_t == 0:
                            nc.scalar.activation(
                                out=a1blk, in_=ps[:],
                                func=mybir.ActivationFunctionType.Copy)
                        else:
                            tb_ = p_agg.tile([128, D], BF, tag="agg",
                                             name="tb1")
                            nc.scalar.activation(
                                out=tb_[:], in_=ps[:],
                                func=mybir.ActivationFunctionType.Copy)
                            nc.vector.tensor_add(out=a1blk, in0=a1blk,
                                                 in1=tb_[:])
                    elif last_t1[t]:
                        del pending[b]
                        tb_ = p_agg.tile([128, D], BF, tag="agg", name="tb1")
                        nc.scalar.activation(
                            out=tb_[:], in_=ps[:],
                            func=mybir.ActivationFunctionType.Copy)
                        aggbf = p_agg.tile([128, D], BF, tag="agg")
                        nc.vector.tensor_add(out=aggbf[:],
                                             in0=acc1[:, b * 128:(b + 1) * 128],
                                             in1=tb_[:])
                        ps2 = p_dps.tile([128, D], f32, space="PSUM", tag="dps")
                        nc.tensor.matmul(out=ps2[:], lhsT=aggbf[:], rhs=W1_t[:],
                                         start=True, stop=False)
                        nc.tensor.matmul(
                            out=ps2[:], lhsT=xts_t[:, b * 128:(b + 1) * 128],
                            rhs=W1_t[:], start=False, stop=True)
                        b5 = b % SUB
                        if b5 == 0:
                            stage1[0] = p_o1.tile([128, SUB * D], BF,
                                                  tag="o1", name="o1w")
                        o1 = stage1[0][:, b5 * D:(b5 + 1) * D]
                        if b1nz:
                            tmp = p_tmp.tile([128, D], f32, tag="tmp")
                            nc.vector.tensor_add(out=tmp[:], in0=ps2[:],
                                                 in1=b1_t[:])
                            nc.scalar.activation(
                                out=o1, in_=tmp[:],
                                func=mybir.ActivationFunctionType.Relu)
                        else:
                            nc.scalar.activation(
                                out=o1, in_=ps2[:],
                                func=mybir.ActivationFunctionType.Relu)
                        c = b // CBLK
                        if b5 == SUB - 1:
                            sl = (b % CBLK) // SUB
                            nc.sync.dma_start(
                                out=ob[c][sl * SUB * 128:(sl + 1) * SUB * 128,
                                          :].rearrange("(j r) f -> r j f",
                                                       j=SUB),
                                in_=stage1[0][:])
                        # mark chunk c ready; the AllGather is emitted after
                        # the NEXT chunk's gather calls so the Pool queue
                        # keeps generating descriptors while AG_c waits on
                        # the bounce writes
                        if (b % CBLK) == CBLK - 1 and variant == "full":
                            assert not chunk_done_emitted[c]
                            chunk_done_emitted[c] = True
                            ag_ready.append(c)

            if variant == "full":
                emit_ags()

            # ---------------- layer 2 ----------------
            pending2 = {}
            for ci, (c, t0, nt) in enumerate(calls2):
                msg_t = p_msg.tile([128, CHUNK_T, D], BF, tag="msg")
                src_ap = hf[c][:] if variant == "full" else x_d[0:CBLK * 128 * NCORES, :]
                nc.gpsimd.dma_gather(
                    out_ap=msg_t[:, :nt, :],
                    in_ap=src_ap,
                    idxs_ap=idx2_t[:, t0 * 8:(t0 + nt) * 8],
                    num_idxs=nt * 128, num_idxs_reg=nt * 128,
                    elem_size=D, queue_num=ci % QUEUES)
                for t in range(t0, t0 + nt):
                    b = int(blk_of_t2[t])
                    S_t = build_S(md2_t, mn2_t, t)
                    if first_t2[t]:
                        pending2[b] = p_eps.tile([128, D], f32, space="PSUM",
                                                 tag="eps", name="eps")
                    ps = pending2[b]
                    nc.tensor.matmul(out=ps[:], lhsT=msg_t[:, t - t0, :],
                                     rhs=S_t[:], start=bool(first_t2[t]),
                                     stop=bool(last_t2[t]))
                    if last_t2[t]:
                        del pending2[b]
                        accblk = acc2[:, b * 128:(b + 1) * 128]
                        if c == 0:
                            nc.scalar.activation(
                                out=accblk, in_=ps[:],
                                func=mybir.ActivationFunctionType.Copy)
                        elif c < NCHUNK - 1:
                            # ACT evicts PSUM to bf16 so the DVE add runs in
                            # the all-16-bit fast mode
                            tbf = p_agg.tile([128, D], BF, tag="agg",
                                             name="tbf")
                            nc.scalar.activation(
                                out=tbf[:], in_=ps[:],
                                func=mybir.ActivationFunctionType.Copy)
                            nc.vector.tensor_add(out=accblk, in0=accblk,
                                                 in1=tbf[:])
                        else:
                            tbf = p_agg.tile([128, D], BF, tag="agg",
                                             name="tbf")
                            nc.scalar.activation(
                                out=tbf[:], in_=ps[:],
                                func=mybir.ActivationFunctionType.Copy)
                            agg2 = p_agg.tile([128, D], BF, tag="agg")
                            nc.vector.tensor_add(out=agg2[:], in0=accblk,
                                                 in1=tbf[:])
                            ps2 = p_dps.tile([128, D], f32, space="PSUM",
                                             tag="dps")
                            nc.tensor.matmul(out=ps2[:], lhsT=agg2[:],
                                             rhs=W2_t[:], start=True, stop=True)
                            b5 = b % SUB
                            if b5 == 0:
                                stage2[0] = p_o1.tile([128, SUB * D], BF,
                                                      tag="o1", name="otw")
                            ot = stage2[0][:, b5 * D:(b5 + 1) * D]
                            if b2nz:
                                tmp = p_tmp.tile([128, D], f32, tag="tmp")
                                nc.vector.tensor_add(out=tmp[:], in0=ps2[:],
                                                     in1=b2_t[:])
                                nc.scalar.activation(
                                    out=ot, in_=tmp[:],
                                    func=mybir.ActivationFunctionType.Relu)
                            else:
                                nc.scalar.activation(
                                    out=ot, in_=ps2[:],
                                    func=mybir.ActivationFunctionType.Relu)
                            if b5 == SUB - 1:
                                nc.sync.dma_start(
                                    out=out_d[(b - SUB + 1) * 128:
                                              (b + 1) * 128, :].rearrange(
                                        "(j r) f -> r j f", j=SUB),
                                    in_=stage2[0][:])

    nc.compile()
    return nc


def prepare(x, edge_index, W1, b1, W2, b2, variant="full"):
    in_maps, sched, pos_of_node = _host_prep(x, edge_index, W1, b1, W2, b2)
    key = (sched["T1"], sched["T2"], sched["calls1"], sched["calls2"],
           sched["b1nz"], sched["b2nz"], variant, PADNEG, QUEUES, SCRATCH)
    if key not in _CACHE:
        _CACHE[key] = _build_program(sched, variant)
    return _CACHE[key], in_maps, pos_of_node


def kernel(x, edge_index, W1, b1, W2, b2):
    nc, in_maps, pos_of_node = prepare(x, edge_index, W1, b1, W2, b2)
    res = run_bass_kernel_spmd(nc, in_maps, list(range(NCORES)))
    full = np.concatenate([res.results[k]["out"] for k in range(NCORES)], axis=0)
    n = np.asarray(x).shape[0]
    return np.ascontiguousarray(full[pos_of_node[:n]].astype(np.float32))
